# revision 13
# baseline (speedup 1.0000x reference)
"""Distributed Bass kernel for causal MHA block (B=4,T=2048,C=1024,H=16,D=64).

Sharding: tensor-parallel over head pairs across 8 cores. Core c owns heads
{2c, 2c+1} and computes QKV+attention for all batches for those heads. The
normalized attention outputs (attnT: head-dims on partitions, tokens free)
are AllGather'd per batch; each core then computes the o-projection for its
128 output channels over all tokens (w_o row-sharded), fused with the
residual add. Host reassembles out = concat(outT_c).T.

Performance structure (v2):
  - All projection matmuls (QKV, V, O) and the PV matmul run in fp8 with
    perf_mode=DoubleRow: contraction pairs are packed 2-per-partition, which
    roughly halves PE streaming time. Scores stay bf16 (D=64 contraction
    cannot pair), but the two heads' score matmuls occupy disjoint PE row
    groups (tile_position via base partitions 0/64) and run concurrently.
  - probs (pt) are e5m2 (dynamic range to 57344, so exp needs no bias and
    cannot flush a whole softmax row to zero); V / activations are e4m3.
  - exp is range-restricted on diagonal blocks to the causal area; the
    128x128 in-block triangle is masked with a single tri tile; dead zones
    of pt that the PV pair would stream are memset to zero.
  - scores are computed transposed (keys on partitions, queries free) so the
    probs tile feeds the PV matmul directly as the moving operand.
  - V carries a fused ones-column; the PV matmul then emits the softmax
    denominator as row 64 of the accumulator for free.
"""

import os
import sys

import numpy as np

sys.path.insert(0, "/opt/trn_rl_repo")

B, T, C, H, D = 4, 2048, 1024, 16, 64
BT = B * T  # 8192
N_CORES = 8
VG = 160  # V_sb per-token-block group: [Ve(64)|1|pad|Vo(64)@80|1@144|pad]

_cache = {}


def _build_graph():
    import concourse.bacc as bacc
    import concourse.bass as bass
    import concourse.mybir as mybir
    import concourse.tile as tile

    f16 = mybir.dt.float16
    bf16 = mybir.dt.bfloat16
    f32 = mybir.dt.float32
    f8e4 = mybir.dt.float8e4
    f8e5 = mybir.dt.float8e5
    Alu = mybir.AluOpType
    Act = mybir.ActivationFunctionType
    DR = mybir.MatmulPerfMode.DoubleRow

    nc = bacc.Bacc("TRN2", target_bir_lowering=False, debug=False,
                   num_devices=N_CORES)

    xT = nc.dram_tensor("xT", [C, BT], f8e4, kind="ExternalInput")
    wqkvT = nc.dram_tensor("wqkvT", [C, 384], f8e4, kind="ExternalInput")
    woT = nc.dram_tensor("woT", [C, 128], f8e4, kind="ExternalInput")
    residT = nc.dram_tensor("residT", [128, BT], f16, kind="ExternalInput")
    tri = nc.dram_tensor("tri", [128, 128], f8e5, kind="ExternalInput")
    outT = nc.dram_tensor("outT", [128, BT], f16, kind="ExternalOutput")

    RG = [list(range(N_CORES))]

    with tile.TileContext(nc) as tc:
        with (
            tc.tile_pool(name="const", bufs=1) as constp,
            tc.tile_pool(name="dram", bufs=1, space="DRAM") as dramp,
            tc.tile_pool(name="qkvout", bufs=1) as qkvp,
            tc.tile_pool(name="ps_st", bufs=2, space="PSUM") as ps_st,
            tc.tile_pool(name="ps_x", bufs=2, space="PSUM") as ps_x,
            tc.tile_pool(name="ps_at", bufs=2, space="PSUM") as ps_at,
        ):
            # ---- constants (wqkvT first: it gates the first matmul) ----
            wqkvT_sb = constp.tile([128, 8 * 384], f8e4)
            for ci in range(8):
                nc.sync.dma_start(out=wqkvT_sb[:, ci * 384:(ci + 1) * 384],
                                  in_=wqkvT[ci * 128:(ci + 1) * 128, :])
            tri_sb = constp.tile([128, 128], f8e5)
            nc.scalar.dma_start(out=tri_sb[:], in_=tri[:])
            woT_sb = constp.tile([128, 8 * 128], f8e4)
            for ci in range(8):
                nc.scalar.dma_start(out=woT_sb[:, ci * 128:(ci + 1) * 128],
                                    in_=woT[ci * 128:(ci + 1) * 128, :])
            wq_r = wqkvT_sb.rearrange("p (c k) -> p c k", k=384)
            wo_r = woT_sb.rearrange("p (c k) -> p c k", k=128)

            # ---- persistent QKV outputs ----
            QT_sb = qkvp.tile([128, BT], bf16)    # rows 0:64 head even, 64:128 odd
            KT_sb = qkvp.tile([128, BT], bf16)
            V_sb = qkvp.tile([128, 64 * VG], f8e4)
            V_g = V_sb.rearrange("p (t g) -> p t g", g=VG)
            nc.vector.memset(V_g[:, :, 64:65], 1.0)
            nc.vector.memset(V_g[:, :, 144:145], 1.0)

            # ---- ag buffers (per batch-half; last batch splits its second
            # half into per-qt collectives to shrink the serial tail) ----
            ag_in = [[dramp.tile([128, 1024], f8e4, name=f"ag_in{b}_{h}")
                      for h in range(2)] for b in range(B)]
            ag_out = [[dramp.tile([1024, 1024], f8e4, name=f"ag_out{b}_{h}",
                                  addr_space="Shared") for h in range(2)]
                      for b in range(B)]
            ag_in3 = [dramp.tile([128, 512], f8e4, name=f"ag_in3_{q}")
                      for q in range(2)]
            ag_out3 = [dramp.tile([1024, 512], f8e4, name=f"ag_out3_{q}",
                                  addr_space="Shared") for q in range(2)]

            with tc.tile_pool(name="xT", bufs=2) as xtp:
                with (
                    tc.tile_pool(name="pt", bufs=6) as ptp,
                    tc.tile_pool(name="rc", bufs=3) as rcp,
                    tc.tile_pool(name="rbs", bufs=2) as rbsp,
                    tc.tile_pool(name="ats", bufs=3) as atsp,
                    tc.tile_pool(name="af", bufs=2) as afp,
                    tc.tile_pool(name="res", bufs=3) as resp,
                    tc.tile_pool(name="os", bufs=4) as osp,
                ):
                    resid_sb = {}
                    xt_sb = {}

                    # ---- emission units -------------------------------
                    # The PE executes its instruction stream in order, so
                    # pure-PE work (QKV projection of the next batch, o-proj
                    # of the previous batch) is chopped into small units and
                    # interleaved into the exp-paced attention stream, where
                    # the PE would otherwise idle waiting on ScalarE.

                    def make_qkv_units(b):
                        tb = b * T
                        units = []

                        def dmas(b=b, tb=tb):
                            xt = xtp.tile([128, 8 * T], f8e4, name="xt")
                            xt_sb[b] = xt
                            if b == 0:
                                # first batch gates everything: fetch the
                                # first 512 tokens of every C-chunk first,
                                # splitting descriptor generation SP/ACT
                                for ci in range(8):
                                    eng = nc.scalar if ci % 2 else nc.sync
                                    eng.dma_start(
                                        out=xt[:, ci * T:ci * T + 512],
                                        in_=xT[ci * 128:(ci + 1) * 128,
                                               tb:tb + 512])
                                for ci in range(8):
                                    eng = nc.scalar if ci % 2 else nc.sync
                                    eng.dma_start(
                                        out=xt[:, ci * T + 512:(ci + 1) * T],
                                        in_=xT[ci * 128:(ci + 1) * 128,
                                               tb + 512:tb + T])
                            else:
                                # prefetched a batch ahead: whole C-chunk
                                # rows (2KB/partition lines, 8 descriptors)
                                for ci in range(8):
                                    nc.sync.dma_start(
                                        out=xt[:, ci * T:(ci + 1) * T],
                                        in_=xT[ci * 128:(ci + 1) * 128,
                                               tb:tb + T])
                            res = resp.tile([128, T], f16, name="res")
                            resid_sb[b] = res
                            eng = nc.scalar if b == 0 else nc.sync
                            eng.dma_start(out=res[:],
                                          in_=residT[:, tb:tb + T])
                        units.append(dmas)

                        state = {}
                        for which, dstname in ((0, "q"), (1, "k")):
                            for tt in range(4):
                                def sub1(b=b, tb=tb, which=which, tt=tt):
                                    ps = ps_x.tile([128, 512], f32, name="x")
                                    state[(which, tt)] = ps
                                    xt_r = xt_sb[b].rearrange(
                                        "p (c t) -> p c t", t=T)
                                    for ci in (0, 2):
                                        nc.tensor.matmul(
                                            ps[:],
                                            wq_r[:, ci:ci + 2,
                                                 which * 128:which * 128 + 128],
                                            xt_r[:, ci:ci + 2,
                                                 tt * 512:(tt + 1) * 512],
                                            start=(ci == 0), stop=False,
                                            perf_mode=DR)

                                def sub2(b=b, tb=tb, which=which, tt=tt):
                                    ps = state.pop((which, tt))
                                    xt_r = xt_sb[b].rearrange(
                                        "p (c t) -> p c t", t=T)
                                    for ci in (4, 6):
                                        nc.tensor.matmul(
                                            ps[:],
                                            wq_r[:, ci:ci + 2,
                                                 which * 128:which * 128 + 128],
                                            xt_r[:, ci:ci + 2,
                                                 tt * 512:(tt + 1) * 512],
                                            start=False, stop=(ci == 6),
                                            perf_mode=DR)
                                    dst = QT_sb if which == 0 else KT_sb
                                    nc.vector.tensor_copy(
                                        dst[:, tb + tt * 512:tb + (tt + 1) * 512],
                                        ps[:])
                                units.append(sub1)
                                units.append(sub2)
                        for vt in range(16):
                            def vu(b=b, tb=tb, vt=vt):
                                t64 = b * 16 + vt
                                ps = ps_x.tile([128, 128], f32, name="x")
                                xt_r = xt_sb[b].rearrange(
                                    "p (c t) -> p c t", t=T)
                                for ci in (0, 2, 4, 6):
                                    nc.tensor.matmul(
                                        ps[:],
                                        xt_r[:, ci:ci + 2,
                                             vt * 128:(vt + 1) * 128],
                                        wq_r[:, ci:ci + 2, 256:384],
                                        start=(ci == 0), stop=(ci == 6),
                                        perf_mode=DR)
                                pv = ps.rearrange("p (h e) -> p h e", e=64)
                                dv = V_g[:, t64, :].rearrange(
                                    "p (h e) -> p h e", e=80)[:, :, 0:64]
                                nc.vector.tensor_copy(dv, pv)
                            units.append(vu)
                        return units

                    def make_oproj_parts(b):
                        tb = b * T
                        parts = []
                        opstate = {}
                        for qt in range(4):
                            def af_fn(b=b, qt=qt):
                                af = afp.tile([128, 8 * 512], f8e4, name="af")
                                if b == B - 1 and qt >= 2:
                                    src = ag_out3[qt - 2].rearrange(
                                        "(c p) q -> p c q", p=128)
                                    nc.sync.dma_start(
                                        out=af.rearrange("p (c q) -> p c q",
                                                         q=512),
                                        in_=src[:, :, :])
                                else:
                                    src = ag_out[b][qt // 2].rearrange(
                                        "(c p) q -> p c q", p=128)
                                    nc.sync.dma_start(
                                        out=af.rearrange("p (c q) -> p c q",
                                                         q=512),
                                        in_=src[:, :, (qt % 2) * 512:
                                                (qt % 2) * 512 + 512])
                                opstate[qt] = af

                            def mm_fn(b=b, tb=tb, qt=qt):
                                af = opstate.pop(qt)
                                af_r = af.rearrange("p (c q) -> p c q", q=512)
                                ps = ps_x.tile([128, 512], f32, name="x")
                                for ci in (0, 2, 4, 6):
                                    nc.tensor.matmul(
                                        ps[:],
                                        wo_r[:, ci:ci + 2, :],
                                        af_r[:, ci:ci + 2, :],
                                        start=(ci == 0), stop=(ci == 6),
                                        perf_mode=DR)
                                osb = osp.tile([128, 512], f16, name="os")
                                nc.vector.tensor_add(
                                    osb[:], ps[:],
                                    resid_sb[b][:, qt * 512:(qt + 1) * 512])
                                nc.sync.dma_start(
                                    out=outT[:, tb + qt * 512:
                                             tb + (qt + 1) * 512],
                                    in_=osb[:])
                            parts.append((af_fn, mm_fn))
                        return parts

                    def make_oproj_units(b):
                        units = []
                        for af_fn, mm_fn in make_oproj_parts(b):
                            def u(af_fn=af_fn, mm_fn=mm_fn):
                                af_fn()
                                mm_fn()
                            units.append(u)
                        return units

                    def emit_attention(b, units, force=None):
                        tb = b * T
                        n_slots = 40
                        total = len(units)
                        popped = 0
                        done_kbs = 0

                        def feed(floor=None):
                            nonlocal popped
                            target = (done_kbs * total + n_slots - 1) // n_slots
                            if floor is not None:
                                target = max(target, floor)
                            while popped < min(target, total):
                                fn, min_kb = units[popped]
                                if min_kb > done_kbs:
                                    break
                                fn()
                                popped += 1

                        for qt in range(4):
                            if force and qt in force:
                                feed(floor=force[qt])
                            q0 = tb + qt * 512
                            nkb = 4 * qt + 4
                            ats = atsp.tile([128, 512], f8e4, name="ats")

                            def n0_of(kb, qt=qt):
                                j = kb - 4 * qt
                                return 128 * j if j > 0 else 0

                            def emit_st(kb, qt=qt, q0=q0, tb=tb):
                                k0 = tb + kb * 128
                                n0 = n0_of(kb)
                                st = ps_st.tile([128, 1024], f32, name="st")
                                for half in (0, 1):
                                    p0 = half * 64
                                    nc.tensor.matmul(
                                        st[:, half * 512 + n0:
                                           half * 512 + 512],
                                        KT_sb[p0:p0 + 64, k0:k0 + 128],
                                        QT_sb[p0:p0 + 64, q0 + n0:q0 + 512],
                                        start=True, stop=True)
                                return st

                            at_eo = [ps_at.tile([65, 512], f32, name="at")
                                     for _ in range(2)]
                            sts = [emit_st(0)]
                            if nkb > 1:
                                sts.append(emit_st(1))
                            pt_r = None
                            for kb in range(nkb):
                                par = kb % 2
                                j = kb - 4 * qt
                                n0 = n0_of(kb)
                                if par == 0:
                                    pt = ptp.tile([128, 2048], f8e5,
                                                  name="pt")
                                    pt_r = pt.rearrange(
                                        "p (h par q) -> p h par q",
                                        par=2, q=512)
                                    # zero the zone the PV pair will stream
                                    # but exp won't write (odd diag blocks)
                                    jo = j + 1
                                    if jo in (1, 3):
                                        zn = 128 * jo
                                        nc.vector.memset(
                                            pt_r[:, :, 1, zn - 128:zn], 0.0)
                                st = sts[kb]
                                st_r = st.rearrange("p (h q) -> p h q", q=512)
                                nc.scalar.activation(
                                    pt_r[:, :, par, n0:512],
                                    st_r[:, :, n0:512], Act.Exp, scale=0.125)
                                if j >= 0:
                                    for hh in (0, 1):
                                        nc.vector.tensor_mul(
                                            pt_r[:, hh, par, n0:n0 + 128],
                                            pt_r[:, hh, par, n0:n0 + 128],
                                            tri_sb[:])
                                if kb + 2 < nkb:
                                    sts.append(emit_st(kb + 2))
                                if par == 1:
                                    p_idx = kb // 2
                                    t64 = b * 16 + 2 * p_idx
                                    m0 = 256 if (2 * p_idx - 4 * qt) == 2 else 0
                                    for half in (0, 1):
                                        nc.tensor.matmul(
                                            at_eo[half][0:65, m0:512],
                                            V_g[:, t64:t64 + 2,
                                                half * 80:half * 80 + 65],
                                            pt_r[:, half, :, m0:512],
                                            start=(p_idx == 0),
                                            stop=(p_idx == 2 * qt + 1),
                                            perf_mode=DR,
                                            skip_group_check=True)
                                done_kbs += 1
                                feed()
                            # normalize + stage for allgather
                            for half in (0, 1):
                                p0 = half * 64
                                at = at_eo[half]
                                den = rcp.tile([1, 512], f32, name="den")
                                nc.vector.tensor_copy(den[:], at[64:65, :])
                                rc = rcp.tile([1, 512], f32, name="rc")
                                nc.vector.reciprocal_approx_fast(
                                    rc[:], den[:])
                                rbs = rbsp.tile([64, 512], f32, name="rbs")
                                nc.gpsimd.partition_broadcast(rbs[:], rc[:])
                                nc.vector.tensor_mul(
                                    ats[p0:p0 + 64, :], at[0:64, :], rbs[:])
                            if b == B - 1 and qt >= 2:
                                nc.gpsimd.dma_start(out=ag_in3[qt - 2][:],
                                                    in_=ats[:])
                                nc.gpsimd.collective_compute(
                                    "AllGather", Alu.bypass, replica_groups=RG,
                                    ins=[ag_in3[qt - 2].opt()],
                                    outs=[ag_out3[qt - 2].opt()])
                            else:
                                nc.gpsimd.dma_start(
                                    out=ag_in[b][qt // 2][:, (qt % 2) * 512:
                                                          (qt % 2) * 512 + 512],
                                    in_=ats[:])
                                if qt % 2 == 1:
                                    nc.gpsimd.collective_compute(
                                        "AllGather", Alu.bypass,
                                        replica_groups=RG,
                                        ins=[ag_in[b][qt // 2].opt()],
                                        outs=[ag_out[b][qt // 2].opt()])
                        # drain any leftovers
                        while popped < total:
                            units[popped][0]()
                            popped += 1

                    # ---- main schedule --------------------------------
                    # batch 0: emit only the slice of QKV that attention
                    # qt0 needs, feed the rest as units with forced pops at
                    # q-tile boundaries (dependency order). Next-batch xt
                    # DMAs go first in every stream so loads never gate the
                    # batch handoff. o-proj af/mm units carry block floors
                    # so the PE reaches them only after their AllGather had
                    # time to complete.
                    q0units = make_qkv_units(0)
                    for idx in (0, 1, 2, 9, 10, 17, 18, 19, 20):
                        q0units[idx]()
                    rest0 = []
                    for j in (1, 2, 3):
                        rest0 += [q0units[2 * j + 1], q0units[2 * j + 2],
                                  q0units[9 + 2 * j], q0units[10 + 2 * j]]
                        rest0 += q0units[17 + 4 * j:21 + 4 * j]

                    def op_floored(b, floors):
                        parts = make_oproj_parts(b)
                        out = []
                        for qt, (fa, fm) in enumerate(floors):
                            out.append((parts[qt][0], fa))
                            out.append((parts[qt][1], fm))
                        return out

                    OP_PREV = [(1, 2), (3, 5), (16, 18), (20, 22)]
                    OP_SELF = [(24, 27), (29, 31), (35, 37)]

                    for b in range(B):
                        qkv = (make_qkv_units(b + 1) if b + 1 < B else [])
                        if b == 0:
                            # interleave: b1 dmas first, then alternate
                            # rest0 / qkv(b1) so b0's own QKV keeps pace
                            units = [(qkv[0], 0)]
                            qi = 1
                            for r in rest0:
                                units.append((r, 0))
                                if qi < len(qkv):
                                    units.append((qkv[qi], 0))
                                    qi += 1
                            units += [(u, 0) for u in qkv[qi:]]
                            force = {1: 16, 2: 32, 3: 48}
                            emit_attention(b, units, force=force)
                            continue
                        opu = op_floored(b - 1, OP_PREV)
                        merged = []
                        qn = [(u, 0) for u in qkv]
                        qi = 0
                        # positions chosen so floors stay monotone and qkv
                        # units spread across the whole stream
                        sched = [1, 2, 3, 5, 16, 18, 20, 22]
                        oi = 0
                        slot = 0
                        while qi < len(qn) or oi < len(opu):
                            if oi < len(opu) and slot >= sched[oi]:
                                merged.append(opu[oi])
                                oi += 1
                            elif qi < len(qn):
                                merged.append(qn[qi])
                                qi += 1
                                slot += 1
                            else:
                                merged.append(opu[oi])
                                oi += 1
                        if b == B - 1:
                            merged += op_floored(b, OP_SELF + [(None, None)])[:6]
                        emit_attention(b, merged, force=None)
                    tailp = make_oproj_parts(B - 1)
                    tailp[3][0]()
                    tailp[3][1]()
    nc.compile()
    return nc


def _host_shards(residual, x, w_qkv, w_o):
    import ml_dtypes
    E4 = ml_dtypes.float8_e4m3
    E5 = ml_dtypes.float8_e5m2
    xf = np.ascontiguousarray(x.reshape(BT, C).T).astype(E4)  # (C, BT)
    rf = residual.reshape(BT, C).T                          # (C, BT) view
    woT_full = w_o.T                                        # (C, C) view

    # in-block causal triangle: allow key s for local query q when s <= q
    ss = np.arange(128)[:, None]
    qq = np.arange(128)[None, :]
    tri = (ss <= qq).astype(E5)
    tri = np.ascontiguousarray(tri)

    in_maps = []
    for c in range(N_CORES):
        r0, r1 = c * 128, (c + 1) * 128
        wq = w_qkv[r0:r1, :]
        wk = w_qkv[C + r0:C + r1, :]
        wv = w_qkv[2 * C + r0:2 * C + r1, :]
        wqkvT = np.ascontiguousarray(
            np.concatenate([wq.T, wk.T, wv.T], axis=1)).astype(E4)
        in_maps.append({
            "xT": xf,
            "wqkvT": wqkvT,
            "woT": np.ascontiguousarray(woT_full[:, r0:r1]).astype(E4),
            "residT": np.ascontiguousarray(rf[r0:r1, :]),
            "tri": tri,
        })
    return in_maps


def kernel(residual, x, w_qkv, w_o):
    from concourse.bass_utils import run_bass_kernel_spmd

    residual = np.asarray(residual, dtype=np.float16)
    x = np.asarray(x, dtype=np.float16)
    w_qkv = np.asarray(w_qkv, dtype=np.float16)
    w_o = np.asarray(w_o, dtype=np.float16)

    if "nc" not in _cache:
        _cache["nc"] = _build_graph()
    nc = _cache["nc"]

    in_maps = _host_shards(residual, x, w_qkv, w_o)
    res = run_bass_kernel_spmd(nc, in_maps, core_ids=list(range(N_CORES)),
                               trace=bool(os.environ.get("BASS_TRACE")))
    _cache["last_result"] = res
    outT = np.concatenate([res.results[c]["outT"] for c in range(N_CORES)],
                          axis=0)                           # (C, BT)
    return np.ascontiguousarray(outT.T).reshape(B, T, C)


# revision 17
# speedup vs baseline: 1.0338x; 1.0338x over previous
"""Distributed Bass kernel for causal MHA block (B=4,T=2048,C=1024,H=16,D=64).

Sharding: tensor-parallel over head pairs across 8 cores. Core c owns heads
{2c, 2c+1} and computes QKV+attention for all batches for those heads. The
normalized attention outputs (attnT: head-dims on partitions, tokens free)
are AllGather'd per batch; each core then computes the o-projection for its
128 output channels over all tokens (w_o row-sharded), fused with the
residual add. Host reassembles out = concat(outT_c).T.

Performance structure (v2):
  - All projection matmuls (QKV, V, O) and the PV matmul run in fp8 with
    perf_mode=DoubleRow: contraction pairs are packed 2-per-partition, which
    roughly halves PE streaming time. Scores stay bf16 (D=64 contraction
    cannot pair), but the two heads' score matmuls occupy disjoint PE row
    groups (tile_position via base partitions 0/64) and run concurrently.
  - probs (pt) are e5m2 (dynamic range to 57344, so exp needs no bias and
    cannot flush a whole softmax row to zero); V / activations are e4m3.
  - exp is range-restricted on diagonal blocks to the causal area; the
    128x128 in-block triangle is masked with a single tri tile; dead zones
    of pt that the PV pair would stream are memset to zero.
  - scores are computed transposed (keys on partitions, queries free) so the
    probs tile feeds the PV matmul directly as the moving operand.
  - V carries a fused ones-column; the PV matmul then emits the softmax
    denominator as row 64 of the accumulator for free.
"""

import os
import sys

import numpy as np

sys.path.insert(0, "/opt/trn_rl_repo")

B, T, C, H, D = 4, 2048, 1024, 16, 64
BT = B * T  # 8192
N_CORES = 8
VG = 160  # V_sb per-token-block group: [Ve(64)|1|pad|Vo(64)@80|1@144|pad]

_cache = {}


def _build_graph():
    import concourse.bacc as bacc
    import concourse.bass as bass
    import concourse.mybir as mybir
    import concourse.tile as tile

    f16 = mybir.dt.float16
    bf16 = mybir.dt.bfloat16
    f32 = mybir.dt.float32
    f8e4 = mybir.dt.float8e4
    f8e5 = mybir.dt.float8e5
    Alu = mybir.AluOpType
    Act = mybir.ActivationFunctionType
    DR = mybir.MatmulPerfMode.DoubleRow

    nc = bacc.Bacc("TRN2", target_bir_lowering=False, debug=False,
                   num_devices=N_CORES)

    xT = nc.dram_tensor("xT", [C, BT], f8e4, kind="ExternalInput")
    wqkvT = nc.dram_tensor("wqkvT", [C, 384], f8e4, kind="ExternalInput")
    woT = nc.dram_tensor("woT", [C, 128], f8e4, kind="ExternalInput")
    residT = nc.dram_tensor("residT", [128, BT], f16, kind="ExternalInput")
    tri = nc.dram_tensor("tri", [128, 128], f8e5, kind="ExternalInput")
    outT = nc.dram_tensor("outT", [128, BT], f16, kind="ExternalOutput")

    RG = [list(range(N_CORES))]

    with tile.TileContext(nc) as tc:
        with (
            tc.tile_pool(name="const", bufs=1) as constp,
            tc.tile_pool(name="dram", bufs=1, space="DRAM") as dramp,
            tc.tile_pool(name="qkvout", bufs=1) as qkvp,
            tc.tile_pool(name="ps_st", bufs=2, space="PSUM") as ps_st,
            tc.tile_pool(name="ps_x", bufs=2, space="PSUM") as ps_x,
            tc.tile_pool(name="ps_at", bufs=2, space="PSUM") as ps_at,
        ):
            # ---- constants (wqkvT first: it gates the first matmul) ----
            wqkvT_sb = constp.tile([128, 8 * 384], f8e4)
            for ci in range(8):
                nc.sync.dma_start(out=wqkvT_sb[:, ci * 384:(ci + 1) * 384],
                                  in_=wqkvT[ci * 128:(ci + 1) * 128, :])
            tri_sb = constp.tile([128, 128], f8e5)
            nc.scalar.dma_start(out=tri_sb[:], in_=tri[:])
            woT_sb = constp.tile([128, 8 * 128], f8e4)
            for ci in range(8):
                nc.scalar.dma_start(out=woT_sb[:, ci * 128:(ci + 1) * 128],
                                    in_=woT[ci * 128:(ci + 1) * 128, :])
            wq_r = wqkvT_sb.rearrange("p (c k) -> p c k", k=384)
            wo_r = woT_sb.rearrange("p (c k) -> p c k", k=128)
            ones_sb = constp.tile([1, 64], f32)
            nc.vector.memset(ones_sb[:], 1.0)
            f32r = mybir.dt.float32r

            # ---- persistent QKV outputs ----
            QT_sb = qkvp.tile([128, BT], bf16)    # rows 0:64 head even, 64:128 odd
            KT_sb = qkvp.tile([128, BT], bf16)
            V_sb = qkvp.tile([128, 64 * VG], f8e4)
            V_g = V_sb.rearrange("p (t g) -> p t g", g=VG)
            nc.vector.memset(V_g[:, :, 64:65], 1.0)
            nc.vector.memset(V_g[:, :, 144:145], 1.0)

            # ---- ag buffers (per batch-half; last batch splits its second
            # half into per-qt collectives to shrink the serial tail) ----
            ag_in = [[dramp.tile([128, 1024], f8e4, name=f"ag_in{b}_{h}")
                      for h in range(2)] for b in range(B)]
            ag_out = [[dramp.tile([1024, 1024], f8e4, name=f"ag_out{b}_{h}",
                                  addr_space="Shared") for h in range(2)]
                      for b in range(B)]
            ag_in3 = [dramp.tile([128, 512], f8e4, name=f"ag_in3_{q}")
                      for q in range(2)]
            ag_out3 = [dramp.tile([1024, 512], f8e4, name=f"ag_out3_{q}",
                                  addr_space="Shared") for q in range(2)]

            with tc.tile_pool(name="xT", bufs=2) as xtp:
                with (
                    tc.tile_pool(name="pt", bufs=6) as ptp,
                    tc.tile_pool(name="rc", bufs=3) as rcp,
                    tc.tile_pool(name="rbs", bufs=2) as rbsp,
                    tc.tile_pool(name="ats", bufs=3) as atsp,
                    tc.tile_pool(name="af", bufs=2) as afp,
                    tc.tile_pool(name="res", bufs=3) as resp,
                    tc.tile_pool(name="os", bufs=4) as osp,
                ):
                    resid_sb = {}
                    xt_sb = {}

                    # ---- emission units -------------------------------
                    # The PE executes its instruction stream in order, so
                    # pure-PE work (QKV projection of the next batch, o-proj
                    # of the previous batch) is chopped into small units and
                    # interleaved into the exp-paced attention stream, where
                    # the PE would otherwise idle waiting on ScalarE.

                    def make_qkv_units(b):
                        tb = b * T
                        units = []

                        def dmas(b=b, tb=tb):
                            xt = xtp.tile([128, 8 * T], f8e4, name="xt")
                            xt_sb[b] = xt
                            if b == 0:
                                # first batch gates everything: fetch the
                                # first 512 tokens of every C-chunk first,
                                # splitting descriptor generation SP/ACT
                                for ci in range(8):
                                    eng = nc.scalar if ci % 2 else nc.sync
                                    eng.dma_start(
                                        out=xt[:, ci * T:ci * T + 512],
                                        in_=xT[ci * 128:(ci + 1) * 128,
                                               tb:tb + 512])
                                for ci in range(8):
                                    eng = nc.scalar if ci % 2 else nc.sync
                                    eng.dma_start(
                                        out=xt[:, ci * T + 512:(ci + 1) * T],
                                        in_=xT[ci * 128:(ci + 1) * 128,
                                               tb + 512:tb + T])
                            else:
                                # prefetched a batch ahead: whole C-chunk
                                # rows (2KB/partition lines, 8 descriptors)
                                for ci in range(8):
                                    nc.sync.dma_start(
                                        out=xt[:, ci * T:(ci + 1) * T],
                                        in_=xT[ci * 128:(ci + 1) * 128,
                                               tb:tb + T])
                            res = resp.tile([128, T], f16, name="res")
                            resid_sb[b] = res
                            eng = nc.scalar if b == 0 else nc.sync
                            eng.dma_start(out=res[:],
                                          in_=residT[:, tb:tb + T])
                        units.append(dmas)

                        state = {}
                        for which, dstname in ((0, "q"), (1, "k")):
                            for tt in range(4):
                                def sub1(b=b, tb=tb, which=which, tt=tt):
                                    ps = ps_x.tile([128, 512], f32, name="x")
                                    state[(which, tt)] = ps
                                    xt_r = xt_sb[b].rearrange(
                                        "p (c t) -> p c t", t=T)
                                    for ci in (0, 2):
                                        nc.tensor.matmul(
                                            ps[:],
                                            wq_r[:, ci:ci + 2,
                                                 which * 128:which * 128 + 128],
                                            xt_r[:, ci:ci + 2,
                                                 tt * 512:(tt + 1) * 512],
                                            start=(ci == 0), stop=False,
                                            perf_mode=DR)

                                def sub2(b=b, tb=tb, which=which, tt=tt):
                                    ps = state.pop((which, tt))
                                    xt_r = xt_sb[b].rearrange(
                                        "p (c t) -> p c t", t=T)
                                    for ci in (4, 6):
                                        nc.tensor.matmul(
                                            ps[:],
                                            wq_r[:, ci:ci + 2,
                                                 which * 128:which * 128 + 128],
                                            xt_r[:, ci:ci + 2,
                                                 tt * 512:(tt + 1) * 512],
                                            start=False, stop=(ci == 6),
                                            perf_mode=DR)
                                    dst = QT_sb if which == 0 else KT_sb
                                    nc.vector.tensor_copy(
                                        dst[:, tb + tt * 512:tb + (tt + 1) * 512],
                                        ps[:])
                                units.append(sub1)
                                units.append(sub2)
                        for vt in range(16):
                            def vu(b=b, tb=tb, vt=vt):
                                t64 = b * 16 + vt
                                ps = ps_x.tile([128, 128], f32, name="x")
                                xt_r = xt_sb[b].rearrange(
                                    "p (c t) -> p c t", t=T)
                                for ci in (0, 2, 4, 6):
                                    nc.tensor.matmul(
                                        ps[:],
                                        xt_r[:, ci:ci + 2,
                                             vt * 128:(vt + 1) * 128],
                                        wq_r[:, ci:ci + 2, 256:384],
                                        start=(ci == 0), stop=(ci == 6),
                                        perf_mode=DR)
                                pv = ps.rearrange("p (h e) -> p h e", e=64)
                                dv = V_g[:, t64, :].rearrange(
                                    "p (h e) -> p h e", e=80)[:, :, 0:64]
                                nc.vector.tensor_copy(dv, pv)
                            units.append(vu)
                        return units

                    def make_oproj_parts(b):
                        tb = b * T
                        parts = []
                        opstate = {}
                        for qt in range(4):
                            def af_fn(b=b, qt=qt):
                                af = afp.tile([128, 8 * 512], f8e4, name="af")
                                if b == B - 1 and qt >= 2:
                                    src = ag_out3[qt - 2].rearrange(
                                        "(c p) q -> p c q", p=128)
                                    nc.sync.dma_start(
                                        out=af.rearrange("p (c q) -> p c q",
                                                         q=512),
                                        in_=src[:, :, :])
                                else:
                                    src = ag_out[b][qt // 2].rearrange(
                                        "(c p) q -> p c q", p=128)
                                    nc.sync.dma_start(
                                        out=af.rearrange("p (c q) -> p c q",
                                                         q=512),
                                        in_=src[:, :, (qt % 2) * 512:
                                                (qt % 2) * 512 + 512])
                                opstate[qt] = af

                            def mm_fn(b=b, tb=tb, qt=qt):
                                af = opstate.pop(qt)
                                af_r = af.rearrange("p (c q) -> p c q", q=512)
                                ps = ps_x.tile([128, 512], f32, name="x")
                                for ci in (0, 2, 4, 6):
                                    nc.tensor.matmul(
                                        ps[:],
                                        wo_r[:, ci:ci + 2, :],
                                        af_r[:, ci:ci + 2, :],
                                        start=(ci == 0), stop=(ci == 6),
                                        perf_mode=DR)
                                osb = osp.tile([128, 512], f16, name="os")
                                nc.vector.tensor_add(
                                    osb[:], ps[:],
                                    resid_sb[b][:, qt * 512:(qt + 1) * 512])
                                nc.sync.dma_start(
                                    out=outT[:, tb + qt * 512:
                                             tb + (qt + 1) * 512],
                                    in_=osb[:])
                            parts.append((af_fn, mm_fn))
                        return parts

                    def make_oproj_units(b):
                        units = []
                        for af_fn, mm_fn in make_oproj_parts(b):
                            def u(af_fn=af_fn, mm_fn=mm_fn):
                                af_fn()
                                mm_fn()
                            units.append(u)
                        return units

                    def emit_attention(b, units, force=None):
                        tb = b * T
                        n_slots = 40
                        total = len(units)
                        popped = 0
                        done_kbs = 0

                        def feed(floor=None):
                            nonlocal popped
                            target = (done_kbs * total + n_slots - 1) // n_slots
                            if floor is not None:
                                target = max(target, floor)
                            while popped < min(target, total):
                                fn, min_kb = units[popped]
                                if min_kb > done_kbs:
                                    break
                                fn()
                                popped += 1

                        for qt in range(4):
                            if force and qt in force:
                                feed(floor=force[qt])
                            q0 = tb + qt * 512
                            nkb = 4 * qt + 4
                            ats = atsp.tile([128, 512], f8e4, name="ats")

                            def n0_of(kb, qt=qt):
                                j = kb - 4 * qt
                                return 128 * j if j > 0 else 0

                            def emit_st(kb, qt=qt, q0=q0, tb=tb):
                                k0 = tb + kb * 128
                                n0 = n0_of(kb)
                                st = ps_st.tile([128, 1024], f32, name="st")
                                for half in (0, 1):
                                    p0 = half * 64
                                    nc.tensor.matmul(
                                        st[:, half * 512 + n0:
                                           half * 512 + 512],
                                        KT_sb[p0:p0 + 64, k0:k0 + 128],
                                        QT_sb[p0:p0 + 64, q0 + n0:q0 + 512],
                                        start=True, stop=True)
                                return st

                            at_eo = [ps_at.tile([65, 512], f32, name="at")
                                     for _ in range(2)]
                            sts = [emit_st(0)]
                            if nkb > 1:
                                sts.append(emit_st(1))
                            pt_r = None
                            for kb in range(nkb):
                                par = kb % 2
                                j = kb - 4 * qt
                                n0 = n0_of(kb)
                                if par == 0:
                                    pt = ptp.tile([128, 2048], f8e5,
                                                  name="pt")
                                    pt_r = pt.rearrange(
                                        "p (h par q) -> p h par q",
                                        par=2, q=512)
                                    # zero the zone the PV pair will stream
                                    # but exp won't write (odd diag blocks)
                                    jo = j + 1
                                    if jo in (1, 3):
                                        zn = 128 * jo
                                        nc.vector.memset(
                                            pt_r[:, :, 1, zn - 128:zn], 0.0)
                                st = sts[kb]
                                st_r = st.rearrange("p (h q) -> p h q", q=512)
                                nc.scalar.activation(
                                    pt_r[:, :, par, n0:512],
                                    st_r[:, :, n0:512], Act.Exp, scale=0.125)
                                if j >= 0:
                                    for hh in (0, 1):
                                        nc.vector.tensor_mul(
                                            pt_r[:, hh, par, n0:n0 + 128],
                                            pt_r[:, hh, par, n0:n0 + 128],
                                            tri_sb[:])
                                if kb + 2 < nkb:
                                    sts.append(emit_st(kb + 2))
                                if par == 1:
                                    p_idx = kb // 2
                                    t64 = b * 16 + 2 * p_idx
                                    m0 = 256 if (2 * p_idx - 4 * qt) == 2 else 0
                                    for half in (0, 1):
                                        nc.tensor.matmul(
                                            at_eo[half][0:65, m0:512],
                                            V_g[:, t64:t64 + 2,
                                                half * 80:half * 80 + 65],
                                            pt_r[:, half, :, m0:512],
                                            start=(p_idx == 0),
                                            stop=(p_idx == 2 * qt + 1),
                                            perf_mode=DR,
                                            skip_group_check=True)
                                done_kbs += 1
                                feed()
                            # normalize + stage for allgather. The partition
                            # broadcast of 1/den runs on the PE (ones-column
                            # outer product, f32r) so the gpsimd queue holds
                            # nothing but collectives — a collective waiting
                            # for peers must not block the next qt's norm.
                            for half in (0, 1):
                                p0 = half * 64
                                at = at_eo[half]
                                den = rcp.tile([1, 512], f32, name="den")
                                nc.vector.tensor_copy(den[:], at[64:65, :])
                                rc = rcp.tile([1, 512], f32, name="rc")
                                nc.vector.reciprocal_approx_fast(
                                    rc[:], den[:])
                                rbs_ps = ps_x.tile([64, 512], f32, name="x")
                                nc.tensor.matmul(
                                    rbs_ps[:], ones_sb[:], rc[:],
                                    start=True, stop=True)
                                rbs = rbsp.tile([64, 512], f32, name="rbs")
                                nc.vector.tensor_copy(rbs[:], rbs_ps[:])
                                nc.vector.tensor_mul(
                                    ats[p0:p0 + 64, :], at[0:64, :], rbs[:])
                            if b == B - 1 and qt >= 2:
                                nc.sync.dma_start(out=ag_in3[qt - 2][:],
                                                  in_=ats[:])
                                nc.gpsimd.collective_compute(
                                    "AllGather", Alu.bypass, replica_groups=RG,
                                    ins=[ag_in3[qt - 2].opt()],
                                    outs=[ag_out3[qt - 2].opt()])
                            else:
                                nc.sync.dma_start(
                                    out=ag_in[b][qt // 2][:, (qt % 2) * 512:
                                                          (qt % 2) * 512 + 512],
                                    in_=ats[:])
                                if qt % 2 == 1:
                                    nc.gpsimd.collective_compute(
                                        "AllGather", Alu.bypass,
                                        replica_groups=RG,
                                        ins=[ag_in[b][qt // 2].opt()],
                                        outs=[ag_out[b][qt // 2].opt()])
                        # drain any leftovers
                        while popped < total:
                            units[popped][0]()
                            popped += 1

                    # ---- main schedule --------------------------------
                    # batch 0: emit only the slice of QKV that attention
                    # qt0 needs, feed the rest as units with forced pops at
                    # q-tile boundaries (dependency order). Next-batch xt
                    # DMAs go first in every stream so loads never gate the
                    # batch handoff. o-proj af/mm units carry block floors
                    # so the PE reaches them only after their AllGather had
                    # time to complete.
                    q0units = make_qkv_units(0)
                    for idx in (0, 1, 2, 9, 10, 17, 18, 19, 20):
                        q0units[idx]()
                    rest0 = []
                    for j in (1, 2, 3):
                        rest0 += [q0units[2 * j + 1], q0units[2 * j + 2],
                                  q0units[9 + 2 * j], q0units[10 + 2 * j]]
                        rest0 += q0units[17 + 4 * j:21 + 4 * j]

                    def op_floored(b, floors):
                        parts = make_oproj_parts(b)
                        out = []
                        for qt, (fa, fm) in enumerate(floors):
                            out.append((parts[qt][0], fa))
                            out.append((parts[qt][1], fm))
                        return out

                    OP_PREV = [(1, 2), (3, 5), (16, 18), (20, 22)]
                    OP_SELF = [(24, 27), (29, 31), (35, 37)]

                    for b in range(B):
                        qkv = (make_qkv_units(b + 1) if b + 1 < B else [])
                        if b == 0:
                            # interleave: b1 dmas first, then alternate
                            # rest0 / qkv(b1) so b0's own QKV keeps pace
                            units = [(qkv[0], 0)]
                            qi = 1
                            for r in rest0:
                                units.append((r, 0))
                                if qi < len(qkv):
                                    units.append((qkv[qi], 0))
                                    qi += 1
                            units += [(u, 0) for u in qkv[qi:]]
                            force = {1: 16, 2: 32, 3: 48}
                            emit_attention(b, units, force=force)
                            continue
                        opu = op_floored(b - 1, OP_PREV)
                        merged = []
                        qn = [(u, 0) for u in qkv]
                        qi = 0
                        # positions chosen so floors stay monotone and qkv
                        # units spread across the whole stream
                        sched = [1, 2, 3, 5, 16, 18, 20, 22]
                        oi = 0
                        slot = 0
                        while qi < len(qn) or oi < len(opu):
                            if oi < len(opu) and slot >= sched[oi]:
                                merged.append(opu[oi])
                                oi += 1
                            elif qi < len(qn):
                                merged.append(qn[qi])
                                qi += 1
                                slot += 1
                            else:
                                merged.append(opu[oi])
                                oi += 1
                        if b == B - 1:
                            merged += op_floored(b, OP_SELF + [(None, None)])[:6]
                        emit_attention(b, merged, force=None)
                    tailp = make_oproj_parts(B - 1)
                    tailp[3][0]()
                    tailp[3][1]()
    nc.compile()
    return nc


def _host_shards(residual, x, w_qkv, w_o):
    import ml_dtypes
    E4 = ml_dtypes.float8_e4m3
    E5 = ml_dtypes.float8_e5m2
    xf = np.ascontiguousarray(x.reshape(BT, C).T).astype(E4)  # (C, BT)
    rf = residual.reshape(BT, C).T                          # (C, BT) view
    woT_full = w_o.T                                        # (C, C) view

    # in-block causal triangle: allow key s for local query q when s <= q
    ss = np.arange(128)[:, None]
    qq = np.arange(128)[None, :]
    tri = (ss <= qq).astype(E5)
    tri = np.ascontiguousarray(tri)

    in_maps = []
    for c in range(N_CORES):
        r0, r1 = c * 128, (c + 1) * 128
        wq = w_qkv[r0:r1, :]
        wk = w_qkv[C + r0:C + r1, :]
        wv = w_qkv[2 * C + r0:2 * C + r1, :]
        wqkvT = np.ascontiguousarray(
            np.concatenate([wq.T, wk.T, wv.T], axis=1)).astype(E4)
        in_maps.append({
            "xT": xf,
            "wqkvT": wqkvT,
            "woT": np.ascontiguousarray(woT_full[:, r0:r1]).astype(E4),
            "residT": np.ascontiguousarray(rf[r0:r1, :]),
            "tri": tri,
        })
    return in_maps


def kernel(residual, x, w_qkv, w_o):
    from concourse.bass_utils import run_bass_kernel_spmd

    residual = np.asarray(residual, dtype=np.float16)
    x = np.asarray(x, dtype=np.float16)
    w_qkv = np.asarray(w_qkv, dtype=np.float16)
    w_o = np.asarray(w_o, dtype=np.float16)

    if "nc" not in _cache:
        _cache["nc"] = _build_graph()
    nc = _cache["nc"]

    in_maps = _host_shards(residual, x, w_qkv, w_o)
    res = run_bass_kernel_spmd(nc, in_maps, core_ids=list(range(N_CORES)),
                               trace=bool(os.environ.get("BASS_TRACE")))
    _cache["last_result"] = res
    outT = np.concatenate([res.results[c]["outT"] for c in range(N_CORES)],
                          axis=0)                           # (C, BT)
    return np.ascontiguousarray(outT.T).reshape(B, T, C)
